# revision 28
# baseline (speedup 1.0000x reference)
"""Trainium2 Bass kernel for nn_Loss_60430189855357.

BCEWithLogits loss + frame metrics over x[32,4,4000,96] @ W[96] + b.

Strategy (data-parallel over batch, 8 cores):
  - host stages x transposed to xt[b,s,f,t] in fp8 e3m4 and W*4 in fp8
    (layout/precision staging only, all FLOPs stay on-chip; measured
    worst-output rel err 8.1e-4 vs the 2e-2 gate). Labels (exact in fp8)
    are packed to the z layout.
  - PE computes every logit: per 125-t chunk, Ldweights(xt[96,125]) +
    Matmult(rhs=4W[96,1]) -> z' column [125,1] in PSUM (z' = 4*W@x).
    512 pairs fill one PSUM bank z'[125, 512], col = (b*4+s)*32 + c,
    partition = t%125; the contraction is fully hidden under the DMA.
  - per (b,s), DVE copies z'+4b to SBUF (sole PSUM reader; PSUM readers
    are serialized by the tile framework, SBUF readers are not), then DVE
    metrics and ACT softplus run in parallel per batch b:
      correct = sum(sum_s((logits>0) != label) < 0.5)   (ne/nesum path)
      FA = sum(label_zero & (max_s logits > 0)),
      MS = sum(~label_zero & (max_s logits <= 0))       (zmax path)
      zy = sum(4*logits*y)/4, softplus via Exp+Ln (preloaded joint table)
  - the x stream (6.1 MB/core fp8) runs gapless at the 360 GB/s DMA
    roofline; output stores ride SP-HWDGE + Pool-SWDGE in parallel.
  - per-core output acc[125, 21]: per-b [correct, FA, MS, 4*z*y] (DVE)
    and 5 softplus partials (ACT; the last batch's first 3 s-blocks are
    hoisted into the stream so only a 32-col Exp+Ln sits in the tail);
    host reduces and applies the reference normalization bit-exactly.
"""

import os
import sys

import numpy as np

if os.path.isdir("/opt/trn_rl_repo") and "/opt/trn_rl_repo" not in sys.path:
    sys.path.insert(0, "/opt/trn_rl_repo")

B, S, T, F = 32, 4, 4000, 96
NCORES = 8
BSH = B // NCORES   # 4 batches per core
P = 125             # z partitions: t offset within a chunk
CH = T // P         # 32 chunks of 125 t per (b, s)
COLS = BSH * S * CH  # 512 z columns = one PSUM bank

TRACE = False          # test.py can flip this to get a profiled run
LAST_RESULT = [None]   # test.py reads BassKernelResults from here


def build_nc(bsh=BSH, s_dim=S, t_dim=T, f_dim=F, p_dim=P):
    import concourse.bacc as bacc
    import concourse.mybir as mybir
    from concourse.tile import TileContext

    ch = t_dim // p_dim
    cols = bsh * s_dim * ch
    dt = mybir.dt
    Alu = mybir.AluOpType
    Ax = mybir.AxisListType
    Act = mybir.ActivationFunctionType

    nc = bacc.Bacc()
    xt_d = nc.declare_dram_parameter("xt", [bsh, s_dim, f_dim, t_dim], dt.float8e3, isOutput=False)
    lab_d = nc.declare_dram_parameter("lab", [p_dim, cols], dt.float8e3, isOutput=False)
    wv_d = nc.declare_dram_parameter("wv", [f_dim, 1], dt.float8e3, isOutput=False)
    bv_d = nc.declare_dram_parameter("bv", [p_dim, 2], dt.float32, isOutput=False)
    acc_d = nc.declare_dram_parameter("acc_out", [p_dim, 21], dt.float32, isOutput=True)

    with (
        TileContext(nc) as tc,
        tc.tile_pool(name="xpool", bufs=8) as px,
        tc.tile_pool(name="mpool", bufs=2) as pm,
        tc.tile_pool(name="apool", bufs=2) as pa,
        tc.tile_pool(name="persist", bufs=1) as pp,
        tc.psum_pool(name="zpool", bufs=1) as pzp,
    ):
        wt = pp.tile([f_dim, 1], dt.float8e3)
        nc.scalar.dma_start(out=wt[:], in_=wv_d[:])
        bt = pp.tile([p_dim, 2], dt.float32)
        nc.scalar.dma_start(out=bt[:], in_=bv_d[:])
        lab8_t = pp.tile([p_dim, cols], dt.float8e3)
        nc.scalar.dma_start(out=lab8_t[:], in_=lab_d[:])
        # one dtype-conversion copy; every consumer reads the fp16 version
        lab_t = pp.tile([p_dim, cols], dt.float16)
        nc.vector.tensor_copy(lab_t[:], lab8_t[:])
        # z' in PSUM is 4*W@x (W pre-scaled by 4 on host for fp8 range);
        # logits = z'/4 + b
        bvec = bt[:, 0:1]    # +b   (ACT softplus bias on the raw z' bank)
        b4vec = bt[:, 1:2]   # +4b  (zb = z' + 4b = 4*logits)
        # preload the joint exp+ln ACT table so the per-b Exp/Ln pairs
        # never reload tables; the load overlaps the x DMA stream
        from concourse.hw_specs import get_activation_tables
        tabs = list(get_activation_tables(nc.m.arch).values())
        set_id = next(i for i, s in enumerate(tabs)
                      if Act.Exp in s and Act.Ln in s)
        nc.scalar.add_instruction(mybir.InstLoadActFuncSet(
            name=nc.get_next_instruction_name(), act_func_set_id=set_id,
            ins=[], outs=[]))

        z_t = pzp.tile([p_dim, cols], dt.float32)
        # DVE-written and ACT-written accumulators are separate tiles so
        # neither engine's queue picks up a cross-engine false dependency
        acc_t = pp.tile([p_dim, 16], dt.float32)
        accsp_t = pp.tile([p_dim, 5], dt.float32)

        # label-only stats, computed as soon as labels arrive (all 4 b):
        # lsum[b-block] = sum_s labels, lz = label_zero
        lsum_all = pp.tile([p_dim, bsh * ch], dt.float16)
        with nc.allow_low_precision(reason="0/1 counts, <=4 terms, exact in fp16"):
            for b in range(bsh):
                sc = s_dim * ch
                nc.vector.tensor_reduce(
                    lsum_all[:, b * ch:(b + 1) * ch],
                    lab_t[:, b * sc:(b + 1) * sc].rearrange(
                        "p (s c) -> p c s", s=s_dim),
                    axis=Ax.X, op=Alu.add)
        lz_all = pp.tile([p_dim, bsh * ch], dt.float16)
        nc.vector.tensor_scalar(lz_all[:], lsum_all[:], 0.5, None, Alu.is_lt)
        tl_all = pp.tile([p_dim, bsh * ch], dt.float16)
        nc.vector.tensor_scalar(tl_all[:], lsum_all[:], 0.5, None, Alu.is_ge)

        for b in range(bsh):
            sc = s_dim * ch
            ls = lab_t[:, b * sc:(b + 1) * sc]
            # zb = z' + 4b = 4*logits, copied PSUM -> SBUF per s-slice; the
            # copy is the SOLE reader of each PSUM range (PSUM readers are
            # serialized by the tile framework, SBUF readers are not), so
            # DVE metrics and ACT softplus then run fully in parallel
            last = b == bsh - 1
            zb_b = pm.tile([p_dim, sc], dt.float32, tag="zb")
            ne_b = pm.tile([p_dim, sc], dt.float16, tag="ne")
            nesum_b = pm.tile([p_dim, ch], dt.float16, tag="nesum")
            zmax_b = pm.tile([p_dim, ch], dt.float32, tag="zmax")

            def compute_cols(s, c0, c1, xtile, xoff):
                """matmuls + zb + ne for z columns [c0, c1) of (b, s); the
                stationary x columns come from xtile starting at t=xoff."""
                base = (b * s_dim + s) * ch
                for c in range(c0, c1):
                    nc.tensor.matmul(
                        out=z_t[:, base + c:base + c + 1],
                        lhsT=xtile[:, c * p_dim - xoff:(c + 1) * p_dim - xoff],
                        rhs=wt[:],
                        start=True, stop=True)
                ssl = slice(s * ch + c0, s * ch + c1)
                nc.vector.tensor_scalar(
                    zb_b[:, ssl], z_t[:, base + c0:base + c1], b4vec, None,
                    Alu.add)
                # ne = (logits > 0) != label, fused
                nc.vector.scalar_tensor_tensor(
                    ne_b[:, ssl], zb_b[:, ssl], 0.0,
                    lab_t[:, b * sc + s * ch + c0:b * sc + s * ch + c1],
                    Alu.is_gt, Alu.not_equal)
                if last:
                    # incremental s-accumulation so the tail after the final
                    # chunk carries one add+max instead of two full reduces
                    if s == 0:
                        nc.vector.tensor_copy(nesum_b[:], ne_b[:, ssl])
                        nc.vector.tensor_copy(zmax_b[:], zb_b[:, ssl])
                    else:
                        with nc.allow_low_precision(
                                reason="0/1 counts, <=4 terms, exact in fp16"):
                            nc.vector.tensor_tensor(
                                nesum_b[:], nesum_b[:], ne_b[:, ssl], Alu.add)
                        nc.vector.tensor_tensor(
                            zmax_b[:], zmax_b[:], zb_b[:, ssl], Alu.max)

            z2_t = None
            for s in range(s_dim):
                xc = px.tile([f_dim, t_dim], dt.float8e3, tag="x")
                # first chunk via SWDGE: its fixed prep latency is lower
                # than the SP HWDGE path, so the stream starts earlier
                if b == 0 and s == 0:
                    nc.gpsimd.dma_start(out=xc[:], in_=xt_d[b, s])
                else:
                    nc.sync.dma_start(out=xc[:], in_=xt_d[b, s])
                if last and s == s_dim - 1:
                    # the PE writes the s3 logits a second time into their
                    # own PSUM bank whose sole reader is ACT, issued BEFORE
                    # the main pairs so softplus starts as early as possible
                    # (PSUM readers serialize; this decouples ACT from the
                    # DVE zb copy)
                    z2_t = pzp.tile([p_dim, ch], dt.float32)
                    for c in range(ch):
                        nc.tensor.matmul(
                            out=z2_t[:, c:c + 1],
                            lhsT=xc[:, c * p_dim:(c + 1) * p_dim],
                            rhs=wt[:],
                            start=True, stop=True)
                compute_cols(s, 0, ch, xc, 0)
                if last and s == s_dim - 2:
                    # softplus for the last batch's first 3 s-blocks rides
                    # the stream; only the s3 block remains in the tail
                    e_a = pa.tile([p_dim, 3 * ch], dt.float32, tag="ea")
                    nc.scalar.activation(
                        e_a[:], zb_b[:, 0:3 * ch], Act.Exp, bias=0.0,
                        scale=0.25)
                    sp_a = pa.tile([p_dim, 3 * ch], dt.float32, tag="spa")
                    nc.scalar.activation(
                        sp_a[:], e_a[:], Act.Ln, bias=1.0,
                        accum_out=accsp_t[:, b:b + 1])

            # softplus = ln(1 + exp(zb/4)) on ACT, parallel with DVE below
            if last:
                e_b = pzp.tile([p_dim, ch], dt.float32)
                nc.scalar.activation(
                    e_b[:], z2_t[:], Act.Exp, bias=bvec, scale=0.25)
                sp_b = pzp.tile([p_dim, ch], dt.float32)
                nc.scalar.activation(
                    sp_b[:], e_b[:], Act.Ln, bias=1.0,
                    accum_out=accsp_t[:, bsh:bsh + 1])
            else:
                e_b = pa.tile([p_dim, sc], dt.float32, tag="eb")
                nc.scalar.activation(
                    e_b[:], zb_b[:], Act.Exp, bias=0.0, scale=0.25)
                sp_b = pa.tile([p_dim, sc], dt.float32, tag="spb")
                nc.scalar.activation(
                    sp_b[:], e_b[:], Act.Ln, bias=1.0,
                    accum_out=accsp_t[:, b:b + 1])

            lzs = lz_all[:, b * ch:(b + 1) * ch]
            lsums = lsum_all[:, b * ch:(b + 1) * ch]

            # pred_zero == all(logits <= 0) == max_s zb <= 0.
            # For label_zero frames, ~match <=> zmax > 0; for label-nonzero
            # pred_zero frames, ~match is guaranteed. So FA and MS need only
            # zmax and the label-only masks, not nesum.
            if not last:
                nc.vector.tensor_reduce(
                    zmax_b[:], zb_b[:].rearrange("p (s c) -> p c s", s=s_dim),
                    axis=Ax.X, op=Alu.max)
            # FA = sum(label_zero * (zmax > 0))
            scr2_b = pm.tile([p_dim, ch], dt.float16, tag="scr2")
            nc.vector.scalar_tensor_tensor(
                scr2_b[:], zmax_b[:], 0.0, lzs, Alu.is_gt, Alu.mult,
                accum_out=acc_t[:, 4 * b + 1:4 * b + 2])
            # MS = sum((lsum >= 0.5) * (zmax <= 0))
            scr3_b = pm.tile([p_dim, ch], dt.float16, tag="scr3")
            nc.vector.scalar_tensor_tensor(
                scr3_b[:], zmax_b[:], 0.0, tl_all[:, b * ch:(b + 1) * ch],
                Alu.is_le, Alu.mult,
                accum_out=acc_t[:, 4 * b + 2:4 * b + 3])

            if not last:
                with nc.allow_low_precision(
                        reason="0/1 counts, <=4 terms, exact in fp16"):
                    nc.vector.tensor_reduce(
                        nesum_b[:], ne_b[:].rearrange("p (s c) -> p c s",
                                                      s=s_dim),
                        axis=Ax.X, op=Alu.add)
            # correct = sum(nesum < 0.5)
            scr_b = pm.tile([p_dim, ch], dt.float16, tag="scr")
            nc.vector.tensor_scalar(
                scr_b[:], nesum_b[:], 0.5, None, Alu.is_lt, Alu.add,
                accum_out=acc_t[:, 4 * b + 0:4 * b + 1])
            # 4*logits*y accumulated; host divides by 4
            zyj_b = pm.tile([p_dim, sc], dt.float32, tag="zyj")
            nc.vector.scalar_tensor_tensor(
                zyj_b[:], zb_b[:], 1.0, ls, Alu.mult, Alu.mult,
                accum_out=acc_t[:, 4 * b + 3:4 * b + 4])

        nc.sync.dma_start(out=acc_d[:, 0:16], in_=acc_t[:])
        # softplus store rides SWDGE (Pool) so its descriptor generation
        # overlaps the SP store's HWDGE prep instead of queuing behind it
        nc.gpsimd.dma_start(out=acc_d[:, 16:21], in_=accsp_t[:])
    nc.finalize()
    return nc


_CACHE = {}


def _get_nc():
    if "nc" not in _CACHE:
        _CACHE["nc"] = build_nc()
    return _CACHE["nc"]


def finalize(sp, zy, correct, FA, MS):
    Ssum = sp - zy
    BT = float(B * T)
    total_loss = Ssum / BT + Ssum / 4.0
    loss = total_loss / BT

    # replicate the reference's sequential fp32 normalization bit-exactly
    f = np.float32
    correct, FA, MS, BT32 = f(correct), f(FA), f(MS), f(BT)
    SC = f(f(f(BT32 - correct) - FA) - MS)
    DER = f(f(f(f(MS + FA) + SC)) / f(f(f(MS + FA) + SC) + correct))
    MS = f(MS / f(f(f(MS + FA) + SC) + correct))
    FA = f(FA / f(f(f(MS + FA) + SC) + correct))
    SC = f(SC / f(f(f(MS + FA) + SC) + correct))
    return (
        np.array(loss, dtype=np.float32),
        np.array(DER, dtype=np.float32),
        np.array(MS, dtype=np.float32),
        np.array(FA, dtype=np.float32),
        np.array(SC, dtype=np.float32),
    )


def kernel(x, labels, W, b):
    from concourse.bass_utils import run_bass_kernel_spmd

    x = np.asarray(x, np.float32)
    labels = np.asarray(labels, np.float32)
    # layout/precision staging (no FLOPs): xt[b,s,f,t] fp8 e3m4, labels
    # packed to the z layout [125, (b s c)]
    from ml_dtypes import float8_e3m4
    xt = np.ascontiguousarray(x.transpose(0, 1, 3, 2)).astype(float8_e3m4)
    lab_re = np.ascontiguousarray(
        labels.reshape(B, S, CH, P).transpose(0, 3, 1, 2)
    ).astype(float8_e3m4)  # [B, 125, S, CH]
    wv = (np.asarray(W, np.float32).reshape(F, 1) * 4.0).astype(float8_e3m4)
    bval = np.float32(np.asarray(b, np.float32).reshape(-1)[0])
    bv = np.empty((P, 2), np.float32)
    bv[:, 0] = bval
    bv[:, 1] = 4.0 * bval

    nc = _get_nc()
    in_maps = []
    for c in range(NCORES):
        lab_c = lab_re[c * BSH:(c + 1) * BSH]  # [BSH, 125, S, CH]
        in_maps.append({
            "xt": xt[c * BSH:(c + 1) * BSH],
            "lab": np.ascontiguousarray(
                lab_c.transpose(1, 0, 2, 3)).reshape(P, COLS),
            "wv": wv,
            "bv": bv,
        })
    res = run_bass_kernel_spmd(nc, in_maps, list(range(NCORES)), trace=TRACE)
    LAST_RESULT[0] = res
    acc = np.stack([np.asarray(r["acc_out"], np.float64) for r in res.results])
    a = acc.sum(axis=(0, 1))  # [20]
    correct = a[0] + a[4] + a[8] + a[12]
    FA = a[1] + a[5] + a[9] + a[13]
    MS = a[2] + a[6] + a[10] + a[14]
    zy = (a[3] + a[7] + a[11] + a[15]) / 4.0
    sp = a[16] + a[17] + a[18] + a[19] + a[20]
    return finalize(sp, zy, correct, FA, MS)


# revision 31
# speedup vs baseline: 1.0046x; 1.0046x over previous
"""Trainium2 Bass kernel for nn_Loss_60430189855357.

BCEWithLogits loss + frame metrics over x[32,4,4000,96] @ W[96] + b.

Strategy (data-parallel over batch, 8 cores):
  - host stages x transposed to xt[b,s,f,t] in fp8 e3m4 and W*4 in fp8
    (layout/precision staging only, all FLOPs stay on-chip; measured
    worst-output rel err 8.1e-4 vs the 2e-2 gate). Labels (exact in fp8)
    are packed to the z layout.
  - PE computes every logit: per 125-t chunk, Ldweights(xt[96,125]) +
    Matmult(rhs=4W[96,1]) -> z' column [125,1] in PSUM (z' = 4*W@x).
    512 pairs fill one PSUM bank z'[125, 512], col = (b*4+s)*32 + c,
    partition = t%125; the contraction is fully hidden under the DMA.
  - per (b,s), DVE copies z'+4b to SBUF (sole PSUM reader; PSUM readers
    are serialized by the tile framework, SBUF readers are not), then DVE
    metrics and ACT softplus run in parallel per batch b:
      correct = sum(sum_s((logits>0) != label) < 0.5)   (ne/nesum path)
      FA = sum(label_zero & (max_s logits > 0)),
      MS = sum(~label_zero & (max_s logits <= 0))       (zmax path)
      zy = sum(4*logits*y)/4, softplus via Exp+Ln (preloaded joint table)
  - the x stream (6.1 MB/core fp8) runs gapless at the 360 GB/s DMA
    roofline; output stores ride SP-HWDGE + Pool-SWDGE in parallel.
  - per-core output acc[125, 21]: per-b [correct, FA, MS, 4*z*y] (DVE)
    and 5 softplus partials (ACT; the last batch's first 3 s-blocks are
    hoisted into the stream so only a 32-col Exp+Ln sits in the tail);
    host reduces and applies the reference normalization bit-exactly.
"""

import os
import sys

import numpy as np

if os.path.isdir("/opt/trn_rl_repo") and "/opt/trn_rl_repo" not in sys.path:
    sys.path.insert(0, "/opt/trn_rl_repo")

B, S, T, F = 32, 4, 4000, 96
NCORES = 8
BSH = B // NCORES   # 4 batches per core
P = 125             # z partitions: t offset within a chunk
CH = T // P         # 32 chunks of 125 t per (b, s)
COLS = BSH * S * CH  # 512 z columns = one PSUM bank

TRACE = False          # test.py can flip this to get a profiled run
LAST_RESULT = [None]   # test.py reads BassKernelResults from here


def build_nc(bsh=BSH, s_dim=S, t_dim=T, f_dim=F, p_dim=P):
    import concourse.bacc as bacc
    import concourse.mybir as mybir
    from concourse.tile import TileContext

    ch = t_dim // p_dim
    cols = bsh * s_dim * ch
    dt = mybir.dt
    Alu = mybir.AluOpType
    Ax = mybir.AxisListType
    Act = mybir.ActivationFunctionType

    nc = bacc.Bacc()
    xt_d = nc.declare_dram_parameter("xt", [bsh, s_dim, f_dim, t_dim], dt.float8e3, isOutput=False)
    lab_d = nc.declare_dram_parameter("lab", [p_dim, cols], dt.float8e3, isOutput=False)
    wv_d = nc.declare_dram_parameter("wv", [f_dim, 1], dt.float8e3, isOutput=False)
    bv_d = nc.declare_dram_parameter("bv", [p_dim, 2], dt.float32, isOutput=False)
    acc_d = nc.declare_dram_parameter("acc_out", [p_dim, 25], dt.float32, isOutput=True)

    with (
        TileContext(nc) as tc,
        tc.tile_pool(name="xpool", bufs=8) as px,
        tc.tile_pool(name="mpool", bufs=2) as pm,
        tc.tile_pool(name="apool", bufs=2) as pa,
        tc.tile_pool(name="persist", bufs=1) as pp,
        tc.psum_pool(name="zpool", bufs=1) as pzp,
    ):
        wt = pp.tile([f_dim, 1], dt.float8e3)
        nc.scalar.dma_start(out=wt[:], in_=wv_d[:])
        bt = pp.tile([p_dim, 2], dt.float32)
        nc.scalar.dma_start(out=bt[:], in_=bv_d[:])
        lab8_t = pp.tile([p_dim, cols], dt.float8e3)
        nc.scalar.dma_start(out=lab8_t[:], in_=lab_d[:])
        # one dtype-conversion copy; every consumer reads the fp16 version
        lab_t = pp.tile([p_dim, cols], dt.float16)
        nc.vector.tensor_copy(lab_t[:], lab8_t[:])
        # z' in PSUM is 4*W@x (W pre-scaled by 4 on host for fp8 range);
        # logits = z'/4 + b
        bvec = bt[:, 0:1]    # +b   (ACT softplus bias on the raw z' bank)
        b4vec = bt[:, 1:2]   # +4b  (zb = z' + 4b = 4*logits)
        # preload the joint exp+ln ACT table so the per-b Exp/Ln pairs
        # never reload tables; the load overlaps the x DMA stream
        from concourse.hw_specs import get_activation_tables
        tabs = list(get_activation_tables(nc.m.arch).values())
        set_id = next(i for i, s in enumerate(tabs)
                      if Act.Exp in s and Act.Ln in s)
        nc.scalar.add_instruction(mybir.InstLoadActFuncSet(
            name=nc.get_next_instruction_name(), act_func_set_id=set_id,
            ins=[], outs=[]))

        z_t = pzp.tile([p_dim, cols], dt.float32)
        # DVE-written and ACT-written accumulators are separate tiles so
        # neither engine's queue picks up a cross-engine false dependency
        acc_t = pp.tile([p_dim, 19], dt.float32)
        accsp_t = pp.tile([p_dim, 6], dt.float32)

        # label-only stats, computed as soon as labels arrive (all 4 b):
        # lsum[b-block] = sum_s labels, lz = label_zero
        lsum_all = pp.tile([p_dim, bsh * ch], dt.float16)
        with nc.allow_low_precision(reason="0/1 counts, <=4 terms, exact in fp16"):
            for b in range(bsh):
                sc = s_dim * ch
                nc.vector.tensor_reduce(
                    lsum_all[:, b * ch:(b + 1) * ch],
                    lab_t[:, b * sc:(b + 1) * sc].rearrange(
                        "p (s c) -> p c s", s=s_dim),
                    axis=Ax.X, op=Alu.add)
        lz_all = pp.tile([p_dim, bsh * ch], dt.float16)
        nc.vector.tensor_scalar(lz_all[:], lsum_all[:], 0.5, None, Alu.is_lt)
        tl_all = pp.tile([p_dim, bsh * ch], dt.float16)
        nc.vector.tensor_scalar(tl_all[:], lsum_all[:], 0.5, None, Alu.is_ge)

        for b in range(bsh):
            sc = s_dim * ch
            ls = lab_t[:, b * sc:(b + 1) * sc]
            # zb = z' + 4b = 4*logits, copied PSUM -> SBUF per s-slice; the
            # copy is the SOLE reader of each PSUM range (PSUM readers are
            # serialized by the tile framework, SBUF readers are not), so
            # DVE metrics and ACT softplus then run fully in parallel
            last = b == bsh - 1
            zb_b = pm.tile([p_dim, sc], dt.float32, tag="zb")
            ne_b = pm.tile([p_dim, sc], dt.float16, tag="ne")
            nesum_b = pm.tile([p_dim, ch], dt.float16, tag="nesum")
            zmax_b = pm.tile([p_dim, ch], dt.float32, tag="zmax")

            def compute_cols(s, c0, c1, xtile, xoff):
                """matmuls + zb + ne for z columns [c0, c1) of (b, s); the
                stationary x columns come from xtile starting at t=xoff."""
                base = (b * s_dim + s) * ch
                for c in range(c0, c1):
                    nc.tensor.matmul(
                        out=z_t[:, base + c:base + c + 1],
                        lhsT=xtile[:, c * p_dim - xoff:(c + 1) * p_dim - xoff],
                        rhs=wt[:],
                        start=True, stop=True)
                ssl = slice(s * ch + c0, s * ch + c1)
                nc.vector.tensor_scalar(
                    zb_b[:, ssl], z_t[:, base + c0:base + c1], b4vec, None,
                    Alu.add)
                # ne = (logits > 0) != label, fused
                nc.vector.scalar_tensor_tensor(
                    ne_b[:, ssl], zb_b[:, ssl], 0.0,
                    lab_t[:, b * sc + s * ch + c0:b * sc + s * ch + c1],
                    Alu.is_gt, Alu.not_equal)
                if last:
                    # incremental s-accumulation so the tail after the final
                    # chunk carries one add+max instead of two full reduces
                    if s == 0:
                        nc.vector.tensor_copy(nesum_b[:], ne_b[:, ssl])
                        nc.vector.tensor_copy(zmax_b[:], zb_b[:, ssl])
                    else:
                        with nc.allow_low_precision(
                                reason="0/1 counts, <=4 terms, exact in fp16"):
                            nc.vector.tensor_tensor(
                                nesum_b[:], nesum_b[:], ne_b[:, ssl], Alu.add)
                        nc.vector.tensor_tensor(
                            zmax_b[:], zmax_b[:], zb_b[:, ssl], Alu.max)
                    # per-s 4*logits*y so only a 32-col op sits in the tail
                    zycol = 4 * b + 3 if s == s_dim - 1 else 16 + s
                    zyj_s = pm.tile([p_dim, ch], dt.float32, tag="zyjs")
                    nc.vector.scalar_tensor_tensor(
                        zyj_s[:], zb_b[:, ssl], 1.0,
                        lab_t[:, b * sc + s * ch + c0:b * sc + s * ch + c1],
                        Alu.mult, Alu.mult,
                        accum_out=acc_t[:, zycol:zycol + 1])

            z2 = {}
            for s in range(s_dim):
                xc = px.tile([f_dim, t_dim], dt.float8e3, tag="x")
                # first chunk via SWDGE: its fixed prep latency is lower
                # than the SP HWDGE path, so the stream starts earlier
                if b == 0 and s == 0:
                    nc.gpsimd.dma_start(out=xc[:], in_=xt_d[b, s])
                else:
                    nc.sync.dma_start(out=xc[:], in_=xt_d[b, s])
                if last and s >= 2:
                    # the PE writes the s2/s3 logits a second time into
                    # their own PSUM banks whose sole reader is ACT, issued
                    # BEFORE the main pairs so softplus starts as early as
                    # possible (PSUM readers serialize; this decouples ACT
                    # from the DVE zb copy)
                    z2s = pzp.tile([p_dim, ch], dt.float32, name=f"z2s{s}")
                    z2[s] = z2s
                    for c in range(ch):
                        nc.tensor.matmul(
                            out=z2s[:, c:c + 1],
                            lhsT=xc[:, c * p_dim:(c + 1) * p_dim],
                            rhs=wt[:],
                            start=True, stop=True)
                compute_cols(s, 0, ch, xc, 0)
                if last and s == 1:
                    # softplus for the last batch's s0-s1 rides the stream
                    # early so the ACT queue is idle when the tail arrives
                    e_a = pa.tile([p_dim, 2 * ch], dt.float32, tag="ea")
                    nc.scalar.activation(
                        e_a[:], zb_b[:, 0:2 * ch], Act.Exp, bias=0.0,
                        scale=0.25)
                    sp_a = pa.tile([p_dim, 2 * ch], dt.float32, tag="spa")
                    nc.scalar.activation(
                        sp_a[:], e_a[:], Act.Ln, bias=1.0,
                        accum_out=accsp_t[:, b:b + 1])
                if last and s == 2:
                    e_a2 = pzp.tile([p_dim, ch], dt.float32)
                    nc.scalar.activation(
                        e_a2[:], z2[2][:], Act.Exp, bias=bvec, scale=0.25)
                    sp_a2 = pzp.tile([p_dim, ch], dt.float32)
                    nc.scalar.activation(
                        sp_a2[:], e_a2[:], Act.Ln, bias=1.0,
                        accum_out=accsp_t[:, bsh + 1:bsh + 2])

            # softplus = ln(1 + exp(zb/4)) on ACT, parallel with DVE below
            if last:
                e_b = pzp.tile([p_dim, ch], dt.float32)
                nc.scalar.activation(
                    e_b[:], z2[3][:], Act.Exp, bias=bvec, scale=0.25)
                sp_b = pzp.tile([p_dim, ch], dt.float32)
                nc.scalar.activation(
                    sp_b[:], e_b[:], Act.Ln, bias=1.0,
                    accum_out=accsp_t[:, bsh:bsh + 1])
            else:
                e_b = pa.tile([p_dim, sc], dt.float32, tag="eb")
                nc.scalar.activation(
                    e_b[:], zb_b[:], Act.Exp, bias=0.0, scale=0.25)
                sp_b = pa.tile([p_dim, sc], dt.float32, tag="spb")
                nc.scalar.activation(
                    sp_b[:], e_b[:], Act.Ln, bias=1.0,
                    accum_out=accsp_t[:, b:b + 1])

            lzs = lz_all[:, b * ch:(b + 1) * ch]
            lsums = lsum_all[:, b * ch:(b + 1) * ch]

            # pred_zero == all(logits <= 0) == max_s zb <= 0.
            # For label_zero frames, ~match <=> zmax > 0; for label-nonzero
            # pred_zero frames, ~match is guaranteed. So FA and MS need only
            # zmax and the label-only masks, not nesum.
            if not last:
                nc.vector.tensor_reduce(
                    zmax_b[:], zb_b[:].rearrange("p (s c) -> p c s", s=s_dim),
                    axis=Ax.X, op=Alu.max)
            # FA = sum(label_zero * (zmax > 0))
            scr2_b = pm.tile([p_dim, ch], dt.float16, tag="scr2")
            nc.vector.scalar_tensor_tensor(
                scr2_b[:], zmax_b[:], 0.0, lzs, Alu.is_gt, Alu.mult,
                accum_out=acc_t[:, 4 * b + 1:4 * b + 2])
            # MS = sum((lsum >= 0.5) * (zmax <= 0))
            scr3_b = pm.tile([p_dim, ch], dt.float16, tag="scr3")
            nc.vector.scalar_tensor_tensor(
                scr3_b[:], zmax_b[:], 0.0, tl_all[:, b * ch:(b + 1) * ch],
                Alu.is_le, Alu.mult,
                accum_out=acc_t[:, 4 * b + 2:4 * b + 3])

            if not last:
                with nc.allow_low_precision(
                        reason="0/1 counts, <=4 terms, exact in fp16"):
                    nc.vector.tensor_reduce(
                        nesum_b[:], ne_b[:].rearrange("p (s c) -> p c s",
                                                      s=s_dim),
                        axis=Ax.X, op=Alu.add)
            # correct = sum(nesum < 0.5)
            scr_b = pm.tile([p_dim, ch], dt.float16, tag="scr")
            nc.vector.tensor_scalar(
                scr_b[:], nesum_b[:], 0.5, None, Alu.is_lt, Alu.add,
                accum_out=acc_t[:, 4 * b + 0:4 * b + 1])
            if not last:
                # 4*logits*y accumulated; host divides by 4
                zyj_b = pm.tile([p_dim, sc], dt.float32, tag="zyj")
                nc.vector.scalar_tensor_tensor(
                    zyj_b[:], zb_b[:], 1.0, ls, Alu.mult, Alu.mult,
                    accum_out=acc_t[:, 4 * b + 3:4 * b + 4])

        nc.sync.dma_start(out=acc_d[:, 0:19], in_=acc_t[:])
        # softplus store rides SWDGE (Pool) so its descriptor generation
        # overlaps the SP store's HWDGE prep instead of queuing behind it
        nc.gpsimd.dma_start(out=acc_d[:, 19:25], in_=accsp_t[:])
    nc.finalize()
    return nc


_CACHE = {}


def _get_nc():
    if "nc" not in _CACHE:
        _CACHE["nc"] = build_nc()
    return _CACHE["nc"]


def finalize(sp, zy, correct, FA, MS):
    Ssum = sp - zy
    BT = float(B * T)
    total_loss = Ssum / BT + Ssum / 4.0
    loss = total_loss / BT

    # replicate the reference's sequential fp32 normalization bit-exactly
    f = np.float32
    correct, FA, MS, BT32 = f(correct), f(FA), f(MS), f(BT)
    SC = f(f(f(BT32 - correct) - FA) - MS)
    DER = f(f(f(f(MS + FA) + SC)) / f(f(f(MS + FA) + SC) + correct))
    MS = f(MS / f(f(f(MS + FA) + SC) + correct))
    FA = f(FA / f(f(f(MS + FA) + SC) + correct))
    SC = f(SC / f(f(f(MS + FA) + SC) + correct))
    return (
        np.array(loss, dtype=np.float32),
        np.array(DER, dtype=np.float32),
        np.array(MS, dtype=np.float32),
        np.array(FA, dtype=np.float32),
        np.array(SC, dtype=np.float32),
    )


def kernel(x, labels, W, b):
    from concourse.bass_utils import run_bass_kernel_spmd

    x = np.asarray(x, np.float32)
    labels = np.asarray(labels, np.float32)
    # layout/precision staging (no FLOPs): xt[b,s,f,t] fp8 e3m4, labels
    # packed to the z layout [125, (b s c)]
    from ml_dtypes import float8_e3m4
    xt = np.ascontiguousarray(x.transpose(0, 1, 3, 2)).astype(float8_e3m4)
    lab_re = np.ascontiguousarray(
        labels.reshape(B, S, CH, P).transpose(0, 3, 1, 2)
    ).astype(float8_e3m4)  # [B, 125, S, CH]
    wv = (np.asarray(W, np.float32).reshape(F, 1) * 4.0).astype(float8_e3m4)
    bval = np.float32(np.asarray(b, np.float32).reshape(-1)[0])
    bv = np.empty((P, 2), np.float32)
    bv[:, 0] = bval
    bv[:, 1] = 4.0 * bval

    nc = _get_nc()
    in_maps = []
    for c in range(NCORES):
        lab_c = lab_re[c * BSH:(c + 1) * BSH]  # [BSH, 125, S, CH]
        in_maps.append({
            "xt": xt[c * BSH:(c + 1) * BSH],
            "lab": np.ascontiguousarray(
                lab_c.transpose(1, 0, 2, 3)).reshape(P, COLS),
            "wv": wv,
            "bv": bv,
        })
    res = run_bass_kernel_spmd(nc, in_maps, list(range(NCORES)), trace=TRACE)
    LAST_RESULT[0] = res
    acc = np.stack([np.asarray(r["acc_out"], np.float64) for r in res.results])
    a = acc.sum(axis=(0, 1))  # [20]
    correct = a[0] + a[4] + a[8] + a[12]
    FA = a[1] + a[5] + a[9] + a[13]
    MS = a[2] + a[6] + a[10] + a[14]
    zy = (a[3] + a[7] + a[11] + a[15] + a[16] + a[17] + a[18]) / 4.0
    sp = a[19] + a[20] + a[21] + a[22] + a[23] + a[24]
    return finalize(sp, zy, correct, FA, MS)


# revision 32
# speedup vs baseline: 1.0076x; 1.0030x over previous
"""Trainium2 Bass kernel for nn_Loss_60430189855357.

BCEWithLogits loss + frame metrics over x[32,4,4000,96] @ W[96] + b.

Strategy (data-parallel over batch, 8 cores):
  - host stages x transposed to xt[b,s,f,t] in fp8 e3m4 and W*4 in fp8
    (layout/precision staging only, all FLOPs stay on-chip; measured
    worst-output rel err 8.1e-4 vs the 2e-2 gate). Labels (exact in fp8)
    are packed to the z layout.
  - PE computes every logit: per 125-t chunk, Ldweights(xt[96,125]) +
    Matmult(rhs=4W[96,1]) -> z' column [125,1] in PSUM (z' = 4*W@x).
    512 pairs fill one PSUM bank z'[125, 512], col = (b*4+s)*32 + c,
    partition = t%125; the contraction is fully hidden under the DMA.
  - per (b,s), DVE copies z'+4b to SBUF (sole PSUM reader; PSUM readers
    are serialized by the tile framework, SBUF readers are not), then DVE
    metrics and ACT softplus run in parallel per batch b:
      correct = sum(sum_s((logits>0) != label) < 0.5)   (ne/nesum path)
      FA = sum(label_zero & (max_s logits > 0)),
      MS = sum(~label_zero & (max_s logits <= 0))       (zmax path)
      zy = sum(4*logits*y)/4, softplus via Exp+Ln (preloaded joint table)
  - the x stream (6.1 MB/core fp8) runs gapless at the 360 GB/s DMA
    roofline; output stores ride SP-HWDGE + Pool-SWDGE in parallel.
  - per-core output acc[125, 21]: per-b [correct, FA, MS, 4*z*y] (DVE)
    and 5 softplus partials (ACT; the last batch's first 3 s-blocks are
    hoisted into the stream so only a 32-col Exp+Ln sits in the tail);
    host reduces and applies the reference normalization bit-exactly.
"""

import os
import sys

import numpy as np

if os.path.isdir("/opt/trn_rl_repo") and "/opt/trn_rl_repo" not in sys.path:
    sys.path.insert(0, "/opt/trn_rl_repo")

B, S, T, F = 32, 4, 4000, 96
NCORES = 8
BSH = B // NCORES   # 4 batches per core
P = 125             # z partitions: t offset within a chunk
CH = T // P         # 32 chunks of 125 t per (b, s)
COLS = BSH * S * CH  # 512 z columns = one PSUM bank

TRACE = False          # test.py can flip this to get a profiled run
LAST_RESULT = [None]   # test.py reads BassKernelResults from here


def build_nc(bsh=BSH, s_dim=S, t_dim=T, f_dim=F, p_dim=P):
    import concourse.bacc as bacc
    import concourse.mybir as mybir
    from concourse.tile import TileContext

    ch = t_dim // p_dim
    cols = bsh * s_dim * ch
    dt = mybir.dt
    Alu = mybir.AluOpType
    Ax = mybir.AxisListType
    Act = mybir.ActivationFunctionType

    nc = bacc.Bacc()
    xt_d = nc.declare_dram_parameter("xt", [bsh, s_dim, f_dim, t_dim], dt.float8e3, isOutput=False)
    lab_d = nc.declare_dram_parameter("lab", [p_dim, cols], dt.float8e3, isOutput=False)
    wv_d = nc.declare_dram_parameter("wv", [f_dim, 1], dt.float8e3, isOutput=False)
    bv_d = nc.declare_dram_parameter("bv", [p_dim, 2], dt.float32, isOutput=False)
    acc_d = nc.declare_dram_parameter("acc_out", [p_dim, 26], dt.float32, isOutput=True)

    with (
        TileContext(nc) as tc,
        tc.tile_pool(name="xpool", bufs=8) as px,
        tc.tile_pool(name="mpool", bufs=2) as pm,
        tc.tile_pool(name="apool", bufs=2) as pa,
        tc.tile_pool(name="persist", bufs=1) as pp,
        tc.psum_pool(name="zpool", bufs=1) as pzp,
    ):
        wt = pp.tile([f_dim, 1], dt.float8e3)
        nc.scalar.dma_start(out=wt[:], in_=wv_d[:])
        bt = pp.tile([p_dim, 2], dt.float32)
        nc.scalar.dma_start(out=bt[:], in_=bv_d[:])
        lab8_t = pp.tile([p_dim, cols], dt.float8e3)
        nc.scalar.dma_start(out=lab8_t[:], in_=lab_d[:])
        # one dtype-conversion copy; every consumer reads the fp16 version
        lab_t = pp.tile([p_dim, cols], dt.float16)
        nc.vector.tensor_copy(lab_t[:], lab8_t[:])
        # z' in PSUM is 4*W@x (W pre-scaled by 4 on host for fp8 range);
        # logits = z'/4 + b
        bvec = bt[:, 0:1]    # +b   (ACT softplus bias on the raw z' bank)
        b4vec = bt[:, 1:2]   # +4b  (zb = z' + 4b = 4*logits)
        # preload the joint exp+ln ACT table so the per-b Exp/Ln pairs
        # never reload tables; the load overlaps the x DMA stream
        from concourse.hw_specs import get_activation_tables
        tabs = list(get_activation_tables(nc.m.arch).values())
        set_id = next(i for i, s in enumerate(tabs)
                      if Act.Exp in s and Act.Ln in s)
        nc.scalar.add_instruction(mybir.InstLoadActFuncSet(
            name=nc.get_next_instruction_name(), act_func_set_id=set_id,
            ins=[], outs=[]))

        z_t = pzp.tile([p_dim, cols], dt.float32)
        # DVE-written and ACT-written accumulators are separate tiles so
        # neither engine's queue picks up a cross-engine false dependency
        acc_t = pp.tile([p_dim, 20], dt.float32)
        accsp_t = pp.tile([p_dim, 6], dt.float32)

        # label-only stats, computed as soon as labels arrive (all 4 b):
        # lsum[b-block] = sum_s labels, lz = label_zero
        lsum_all = pp.tile([p_dim, bsh * ch], dt.float16)
        with nc.allow_low_precision(reason="0/1 counts, <=4 terms, exact in fp16"):
            for b in range(bsh):
                sc = s_dim * ch
                nc.vector.tensor_reduce(
                    lsum_all[:, b * ch:(b + 1) * ch],
                    lab_t[:, b * sc:(b + 1) * sc].rearrange(
                        "p (s c) -> p c s", s=s_dim),
                    axis=Ax.X, op=Alu.add)
        lz_all = pp.tile([p_dim, bsh * ch], dt.float16)
        nc.vector.tensor_scalar(lz_all[:], lsum_all[:], 0.5, None, Alu.is_lt)
        tl_all = pp.tile([p_dim, bsh * ch], dt.float16)
        nc.vector.tensor_scalar(tl_all[:], lsum_all[:], 0.5, None, Alu.is_ge)

        for b in range(bsh):
            sc = s_dim * ch
            ls = lab_t[:, b * sc:(b + 1) * sc]
            # zb = z' + 4b = 4*logits, copied PSUM -> SBUF per s-slice; the
            # copy is the SOLE reader of each PSUM range (PSUM readers are
            # serialized by the tile framework, SBUF readers are not), so
            # DVE metrics and ACT softplus then run fully in parallel
            last = b == bsh - 1
            zb_b = pm.tile([p_dim, sc], dt.float32, tag="zb")
            ne_b = pm.tile([p_dim, sc], dt.float16, tag="ne")
            nesum_b = pm.tile([p_dim, ch], dt.float16, tag="nesum")
            zmax_b = pm.tile([p_dim, ch], dt.float32, tag="zmax")

            def compute_cols(s, c0, c1, xtile, xoff):
                """matmuls + zb + ne for z columns [c0, c1) of (b, s); the
                stationary x columns come from xtile starting at t=xoff."""
                base = (b * s_dim + s) * ch
                for c in range(c0, c1):
                    nc.tensor.matmul(
                        out=z_t[:, base + c:base + c + 1],
                        lhsT=xtile[:, c * p_dim - xoff:(c + 1) * p_dim - xoff],
                        rhs=wt[:],
                        start=True, stop=True)
                ssl = slice(s * ch + c0, s * ch + c1)
                nc.vector.tensor_scalar(
                    zb_b[:, ssl], z_t[:, base + c0:base + c1], b4vec, None,
                    Alu.add)
                # ne = (logits > 0) != label, fused
                nc.vector.scalar_tensor_tensor(
                    ne_b[:, ssl], zb_b[:, ssl], 0.0,
                    lab_t[:, b * sc + s * ch + c0:b * sc + s * ch + c1],
                    Alu.is_gt, Alu.not_equal)
                if last:
                    # incremental s-accumulation so the tail after the final
                    # chunk carries one add+max instead of two full reduces
                    if s == 0:
                        nc.vector.tensor_copy(nesum_b[:], ne_b[:, ssl])
                        nc.vector.tensor_copy(zmax_b[:], zb_b[:, ssl])
                    elif s < s_dim - 1:
                        with nc.allow_low_precision(
                                reason="0/1 counts, <=4 terms, exact in fp16"):
                            nc.vector.tensor_tensor(
                                nesum_b[:], nesum_b[:], ne_b[:, ssl], Alu.add)
                        nc.vector.tensor_tensor(
                            zmax_b[:], zmax_b[:], zb_b[:, ssl], Alu.max)
                    # per-s 4*logits*y so only a 32-col op sits in the tail
                    zycol = 4 * b + 3 if s == s_dim - 1 else 16 + s
                    zyj_s = pm.tile([p_dim, ch], dt.float32, tag="zyjs")
                    nc.vector.scalar_tensor_tensor(
                        zyj_s[:], zb_b[:, ssl], 1.0,
                        lab_t[:, b * sc + s * ch + c0:b * sc + s * ch + c1],
                        Alu.mult, Alu.mult,
                        accum_out=acc_t[:, zycol:zycol + 1])

            z2 = {}
            for s in range(s_dim):
                xc = px.tile([f_dim, t_dim], dt.float8e3, tag="x")
                # first chunk via SWDGE: its fixed prep latency is lower
                # than the SP HWDGE path, so the stream starts earlier
                if b == 0 and s == 0:
                    nc.gpsimd.dma_start(out=xc[:], in_=xt_d[b, s])
                else:
                    nc.sync.dma_start(out=xc[:], in_=xt_d[b, s])
                if last and s >= 2:
                    # the PE writes the s2/s3 logits a second time into
                    # their own PSUM banks whose sole reader is ACT, issued
                    # BEFORE the main pairs so softplus starts as early as
                    # possible (PSUM readers serialize; this decouples ACT
                    # from the DVE zb copy)
                    z2s = pzp.tile([p_dim, ch], dt.float32, name=f"z2s{s}")
                    z2[s] = z2s
                    for c in range(ch):
                        nc.tensor.matmul(
                            out=z2s[:, c:c + 1],
                            lhsT=xc[:, c * p_dim:(c + 1) * p_dim],
                            rhs=wt[:],
                            start=True, stop=True)
                compute_cols(s, 0, ch, xc, 0)
                if last and s == 1:
                    # softplus for the last batch's s0-s1 rides the stream
                    # early so the ACT queue is idle when the tail arrives
                    e_a = pa.tile([p_dim, 2 * ch], dt.float32, tag="ea")
                    nc.scalar.activation(
                        e_a[:], zb_b[:, 0:2 * ch], Act.Exp, bias=0.0,
                        scale=0.25)
                    sp_a = pa.tile([p_dim, 2 * ch], dt.float32, tag="spa")
                    nc.scalar.activation(
                        sp_a[:], e_a[:], Act.Ln, bias=1.0,
                        accum_out=accsp_t[:, b:b + 1])
                if last and s == 2:
                    # fold the s0-s2 metric state into masks now, so the s3
                    # contribution needs one fused STT per metric in the tail
                    lzs3 = lz_all[:, b * ch:(b + 1) * ch]
                    tls3 = tl_all[:, b * ch:(b + 1) * ch]
                    m012_t = pm.tile([p_dim, ch], dt.float16, tag="m012")
                    nc.vector.tensor_scalar(
                        m012_t[:], nesum_b[:], 0.5, None, Alu.is_lt)
                    pz012_t = pm.tile([p_dim, ch], dt.float16, tag="pz012")
                    nc.vector.tensor_scalar(
                        pz012_t[:], zmax_b[:], 0.0, None, Alu.is_le)
                    q_t = pm.tile([p_dim, ch], dt.float16, tag="qt")
                    nc.vector.tensor_tensor(q_t[:], lzs3, pz012_t[:], Alu.mult)
                    r_t = pm.tile([p_dim, ch], dt.float16, tag="rt")
                    nc.vector.tensor_tensor(r_t[:], tls3, pz012_t[:], Alu.mult)
                    # label-zero frame count for this batch (host: FA = lzsum - FA')
                    lzc_t = pm.tile([p_dim, ch], dt.float16, tag="lzc")
                    nc.vector.tensor_scalar(
                        lzc_t[:], lzs3, 1.0, None, Alu.mult, Alu.add,
                        accum_out=acc_t[:, 19:20])
                    e_a2 = pzp.tile([p_dim, ch], dt.float32)
                    nc.scalar.activation(
                        e_a2[:], z2[2][:], Act.Exp, bias=bvec, scale=0.25)
                    sp_a2 = pzp.tile([p_dim, ch], dt.float32)
                    nc.scalar.activation(
                        sp_a2[:], e_a2[:], Act.Ln, bias=1.0,
                        accum_out=accsp_t[:, bsh + 1:bsh + 2])

            # softplus = ln(1 + exp(zb/4)) on ACT, parallel with DVE below
            if last:
                e_b = pzp.tile([p_dim, ch], dt.float32)
                nc.scalar.activation(
                    e_b[:], z2[3][:], Act.Exp, bias=bvec, scale=0.25)
                sp_b = pzp.tile([p_dim, ch], dt.float32)
                nc.scalar.activation(
                    sp_b[:], e_b[:], Act.Ln, bias=1.0,
                    accum_out=accsp_t[:, bsh:bsh + 1])
            else:
                e_b = pa.tile([p_dim, sc], dt.float32, tag="eb")
                nc.scalar.activation(
                    e_b[:], zb_b[:], Act.Exp, bias=0.0, scale=0.25)
                sp_b = pa.tile([p_dim, sc], dt.float32, tag="spb")
                nc.scalar.activation(
                    sp_b[:], e_b[:], Act.Ln, bias=1.0,
                    accum_out=accsp_t[:, b:b + 1])

            lzs = lz_all[:, b * ch:(b + 1) * ch]
            lsums = lsum_all[:, b * ch:(b + 1) * ch]

            # pred_zero == all(logits <= 0) == max_s zb <= 0.
            # For label_zero frames, ~match <=> zmax > 0; for label-nonzero
            # pred_zero frames, ~match is guaranteed. So FA and MS need only
            # zmax and the label-only masks, not nesum.
            if last:
                s3 = slice(3 * ch, 4 * ch)
                # FA' = sum(q * (zb_s3 <= 0)); host: FA = lzsum - FA'
                scr2_b = pm.tile([p_dim, ch], dt.float16, tag="scr2")
                nc.vector.scalar_tensor_tensor(
                    scr2_b[:], zb_b[:, s3], 0.0, q_t[:], Alu.is_le, Alu.mult,
                    accum_out=acc_t[:, 4 * b + 1:4 * b + 2])
                # MS = sum(r * (zb_s3 <= 0))
                scr3_b = pm.tile([p_dim, ch], dt.float16, tag="scr3")
                nc.vector.scalar_tensor_tensor(
                    scr3_b[:], zb_b[:, s3], 0.0, r_t[:], Alu.is_le, Alu.mult,
                    accum_out=acc_t[:, 4 * b + 2:4 * b + 3])
                # correct = sum(m012 * (ne_s3 < 0.5))
                scr_b = pm.tile([p_dim, ch], dt.float16, tag="scr")
                nc.vector.scalar_tensor_tensor(
                    scr_b[:], ne_b[:, s3], 0.5, m012_t[:], Alu.is_lt, Alu.mult,
                    accum_out=acc_t[:, 4 * b + 0:4 * b + 1])
            else:
                nc.vector.tensor_reduce(
                    zmax_b[:], zb_b[:].rearrange("p (s c) -> p c s", s=s_dim),
                    axis=Ax.X, op=Alu.max)
                # FA = sum(label_zero * (zmax > 0))
                scr2_b = pm.tile([p_dim, ch], dt.float16, tag="scr2")
                nc.vector.scalar_tensor_tensor(
                    scr2_b[:], zmax_b[:], 0.0, lzs, Alu.is_gt, Alu.mult,
                    accum_out=acc_t[:, 4 * b + 1:4 * b + 2])
                # MS = sum((lsum >= 0.5) * (zmax <= 0))
                scr3_b = pm.tile([p_dim, ch], dt.float16, tag="scr3")
                nc.vector.scalar_tensor_tensor(
                    scr3_b[:], zmax_b[:], 0.0, tl_all[:, b * ch:(b + 1) * ch],
                    Alu.is_le, Alu.mult,
                    accum_out=acc_t[:, 4 * b + 2:4 * b + 3])
                with nc.allow_low_precision(
                        reason="0/1 counts, <=4 terms, exact in fp16"):
                    nc.vector.tensor_reduce(
                        nesum_b[:], ne_b[:].rearrange("p (s c) -> p c s",
                                                      s=s_dim),
                        axis=Ax.X, op=Alu.add)
                # correct = sum(nesum < 0.5)
                scr_b = pm.tile([p_dim, ch], dt.float16, tag="scr")
                nc.vector.tensor_scalar(
                    scr_b[:], nesum_b[:], 0.5, None, Alu.is_lt, Alu.add,
                    accum_out=acc_t[:, 4 * b + 0:4 * b + 1])
            if not last:
                # 4*logits*y accumulated; host divides by 4
                zyj_b = pm.tile([p_dim, sc], dt.float32, tag="zyj")
                nc.vector.scalar_tensor_tensor(
                    zyj_b[:], zb_b[:], 1.0, ls, Alu.mult, Alu.mult,
                    accum_out=acc_t[:, 4 * b + 3:4 * b + 4])

        nc.sync.dma_start(out=acc_d[:, 0:20], in_=acc_t[:])
        # softplus store rides SWDGE (Pool) so its descriptor generation
        # overlaps the SP store's HWDGE prep instead of queuing behind it
        nc.gpsimd.dma_start(out=acc_d[:, 20:26], in_=accsp_t[:])
    nc.finalize()
    return nc


_CACHE = {}


def _get_nc():
    if "nc" not in _CACHE:
        _CACHE["nc"] = build_nc()
    return _CACHE["nc"]


def finalize(sp, zy, correct, FA, MS):
    Ssum = sp - zy
    BT = float(B * T)
    total_loss = Ssum / BT + Ssum / 4.0
    loss = total_loss / BT

    # replicate the reference's sequential fp32 normalization bit-exactly
    f = np.float32
    correct, FA, MS, BT32 = f(correct), f(FA), f(MS), f(BT)
    SC = f(f(f(BT32 - correct) - FA) - MS)
    DER = f(f(f(f(MS + FA) + SC)) / f(f(f(MS + FA) + SC) + correct))
    MS = f(MS / f(f(f(MS + FA) + SC) + correct))
    FA = f(FA / f(f(f(MS + FA) + SC) + correct))
    SC = f(SC / f(f(f(MS + FA) + SC) + correct))
    return (
        np.array(loss, dtype=np.float32),
        np.array(DER, dtype=np.float32),
        np.array(MS, dtype=np.float32),
        np.array(FA, dtype=np.float32),
        np.array(SC, dtype=np.float32),
    )


def kernel(x, labels, W, b):
    from concourse.bass_utils import run_bass_kernel_spmd

    x = np.asarray(x, np.float32)
    labels = np.asarray(labels, np.float32)
    # layout/precision staging (no FLOPs): xt[b,s,f,t] fp8 e3m4, labels
    # packed to the z layout [125, (b s c)]
    from ml_dtypes import float8_e3m4
    xt = np.ascontiguousarray(x.transpose(0, 1, 3, 2)).astype(float8_e3m4)
    lab_re = np.ascontiguousarray(
        labels.reshape(B, S, CH, P).transpose(0, 3, 1, 2)
    ).astype(float8_e3m4)  # [B, 125, S, CH]
    wv = (np.asarray(W, np.float32).reshape(F, 1) * 4.0).astype(float8_e3m4)
    bval = np.float32(np.asarray(b, np.float32).reshape(-1)[0])
    bv = np.empty((P, 2), np.float32)
    bv[:, 0] = bval
    bv[:, 1] = 4.0 * bval

    nc = _get_nc()
    in_maps = []
    for c in range(NCORES):
        lab_c = lab_re[c * BSH:(c + 1) * BSH]  # [BSH, 125, S, CH]
        in_maps.append({
            "xt": xt[c * BSH:(c + 1) * BSH],
            "lab": np.ascontiguousarray(
                lab_c.transpose(1, 0, 2, 3)).reshape(P, COLS),
            "wv": wv,
            "bv": bv,
        })
    res = run_bass_kernel_spmd(nc, in_maps, list(range(NCORES)), trace=TRACE)
    LAST_RESULT[0] = res
    acc = np.stack([np.asarray(r["acc_out"], np.float64) for r in res.results])
    a = acc.sum(axis=(0, 1))  # [20]
    correct = a[0] + a[4] + a[8] + a[12]
    FA = a[1] + a[5] + a[9] + (a[19] - a[13])
    MS = a[2] + a[6] + a[10] + a[14]
    zy = (a[3] + a[7] + a[11] + a[15] + a[16] + a[17] + a[18]) / 4.0
    sp = a[20] + a[21] + a[22] + a[23] + a[24] + a[25]
    return finalize(sp, zy, correct, FA, MS)
